# revision 32
# baseline (speedup 1.0000x reference)
"""AttentionBlock (GroupNorm + single-head self-attention + residual) on 8 TRN2 cores.

Strategy: pure data-parallel over batch (16 items -> 2 per core), no collectives.
All six big matmul groups run in fp8 (TRN FP8_EXP4, max +-240) with
perf_mode=DoubleRow: each PE cell holds 2 fp8 weights, virtualizing the array
to 256 contraction rows per pass (~1.5x bf16 throughput at FD=512).
Operand tiles keep the natural [128, KS, free] block layout; a DR matmul
consumes a [:, ks:ks+2, m] 3D slice of lhsT and [:, ks:ks+2, n] of rhs
(contraction k = (ks+ko)*128 + p), so evictions write plain contiguous tiles.

fp8 scale bookkeeping (all power-of-2, folded into eviction scales):
  w' = 16w for wq,wk,wv,wp    hn in fp8 unscaled (~N(0,1))
  q' = 16(q+bq), k' = 16(k+bk)   (bias' = 16*bias via ACT Identity)
  logits_ps = q'.k' = 256 q.k -> e = Exp(ps * SCALE/256 - 1)  (shift keeps
      max e ~= 118 < 240; cancels exactly in softmax)
  sums_ps = ones(=4) @ e = 4*sum_j e ;  recipB = bcast(1/sums_ps)
  vT' = 16v ; ou' = (sum_j vT' e)/64 [ACT evict] ; proj_ps = w_p' @ ou'
  o = proj_ps * recipB + (x + bpp)   [two DVE ops at proj evict]
Simulated end-to-end fp8 error vs reference: 5.6e-3 (gate 2e-2).

Scheduling: GroupNorm runs off a bf16 copy of x so the startup-critical DMA
is 1MB, streaming first and alone (the 16 HW DMA engines split bandwidth
over everything in flight, so triggering everything at t=0 starves the
critical path); weight and f32-x triggers ride the idle gpsimd queue behind
a ~5us dummy delay op. proj uses [128,512] psums from the psv pool so
proj(0) never contends with qkv(1) on psq, and evicts+stores per 512-chunk
to keep the post-last-matmul tail to one chunk.
"""

import numpy as np
import ml_dtypes

B_TOT, C, H, W = 16, 512, 32, 32
N = H * W            # 1024
NCORES = 8
BPC = B_TOT // NCORES  # 2 batch items per core
CT = C // 128        # 4 channel tiles
NT = N // 128        # 8 position tiles
NCH = N // 512       # 2 free-dim chunks of 512
GS = 16              # group size (channels per group)
EPS = 1e-5
SCALE = float(C) ** -0.5

_CACHE = {}


def _build_bass():
    import concourse.bass as bass  # noqa: F401
    import concourse.tile as tile
    from concourse import bacc, mybir

    F32 = mybir.dt.float32
    BF16 = mybir.dt.bfloat16
    FP8 = mybir.dt.float8e4
    I32 = mybir.dt.int32
    Alu = mybir.AluOpType
    Act = mybir.ActivationFunctionType
    DR = mybir.MatmulPerfMode.DoubleRow

    nc = bacc.Bacc("TRN2", target_bir_lowering=False, debug=False,
                   num_devices=NCORES)

    x_ext = nc.dram_tensor("x", [BPC, 128, CT, N], F32, kind="ExternalInput").ap()
    xh_ext = nc.dram_tensor("xh", [BPC, 128, CT, N], BF16,
                            kind="ExternalInput").ap()
    w_ext = {
        name: nc.dram_tensor(name, [128, CT, 512], FP8, kind="ExternalInput").ap()
        for name in ("wq", "wk", "wv", "wp")
    }
    vec_ext = {
        name: nc.dram_tensor(name, [128, CT], F32, kind="ExternalInput").ap()
        for name in ("gamma", "beta", "bq", "bk", "bpp")
    }
    sel_ext = nc.dram_tensor("sel", [128, 128], F32, kind="ExternalInput").ap()
    ones_ext = nc.dram_tensor("ones", [128, 2, 16], FP8, kind="ExternalInput").ap()
    out_ext = nc.dram_tensor("out", [BPC, 128, CT, N], F32, kind="ExternalOutput").ap()

    with tile.TileContext(nc) as tc:
        with (
            tc.tile_pool(name="consts", bufs=1) as consts,
            tc.tile_pool(name="xp", bufs=2) as xp,
            tc.tile_pool(name="hnp", bufs=2) as hnp,
            tc.tile_pool(name="qkp", bufs=1) as qkp,
            tc.tile_pool(name="vp", bufs=1) as vp,
            tc.tile_pool(name="ep", bufs=2) as ep,
            tc.tile_pool(name="oup", bufs=1) as oup,
            tc.tile_pool(name="outp", bufs=3) as outp,
            tc.tile_pool(name="xbp", bufs=2) as xbp,
            tc.tile_pool(name="rp", bufs=2) as rp,
            tc.tile_pool(name="smallp", bufs=8) as smallp,
            tc.tile_pool(name="psq", bufs=2, space="PSUM") as psq,
            tc.tile_pool(name="psv", bufs=2, space="PSUM") as psv,
            tc.tile_pool(name="psg", bufs=1, space="PSUM") as psg,
        ):
            # ---- staggered input DMAs ----
            # GroupNorm runs off a bf16 copy of x (0.17% hn error, buried
            # under fp8's ~4%), so the startup-critical DMA is 1MB instead
            # of 4MB. The f32 x (residual path only, needed ~40us later)
            # streams in the background off the gpsimd queue.
            # xh is one consolidated tile per batch, moved as two
            # 4KB-per-partition-line half transfers: these DMAs are
            # descriptor-bound (a 4KB line costs the same as 2KB), so
            # halving the transfer count halves the xh landing time.
            # The sync queue carries ONLY xh0 so it triggers at t~6.5.
            xts = [[None] * CT for _ in range(BPC)]
            xhts = []
            for b in range(BPC):
                xhts.append(xp.tile([128, CT, N], BF16, tag="xh",
                                    name=f"xh_b{b}"))
                for t in range(CT):
                    xts[b][t] = xp.tile([128, N], F32, tag=f"x{t}",
                                        name=f"x_b{b}_t{t}")
            for hf in range(2):
                nc.sync.dma_start(xhts[0][:, 2 * hf:2 * hf + 2, :],
                                  xh_ext[0, :, 2 * hf:2 * hf + 2, :])
            vec_sb = {}
            for name in ("gamma", "beta", "bq", "bk", "bpp"):
                vec_sb[name] = consts.tile([128, CT], F32, tag=name,
                                           name=f"vec_{name}")
                nc.scalar.dma_start(vec_sb[name][:], vec_ext[name][:])
            sel_sb = consts.tile([128, 128], F32, tag="sel")
            nc.scalar.dma_start(sel_sb[:], sel_ext[:])
            ones_sb = consts.tile([128, 2, 16], FP8, tag="ones")
            nc.scalar.dma_start(ones_sb[:], ones_ext[:])

            magic_sb = consts.tile([128, 1], I32, tag="magic")
            nc.vector.memset(magic_sb[:], 0x5F3759DF)
            negone_sb = consts.tile([128, 1], F32, tag="negone")
            nc.vector.memset(negone_sb[:], -1.0)

            # Deferred weight/x1 triggers ride the otherwise-idle gpsimd
            # queue behind a dummy op sized to ~8us (gpsimd runs ~14ns per
            # element-column). The wq DMA is additionally WAR-fenced by a
            # read of its target tile so the Tile scheduler cannot hoist it
            # above the delay (it did exactly that without the fence).
            w_sb = {
                name: consts.tile([128, CT, 512], FP8, tag=name, name=f"w_{name}")
                for name in ("wq", "wk", "wv", "wp")
            }
            dummy_sb = consts.tile([128, 448], F32, tag="dummy")
            junk_sb = consts.tile([128, 64], mybir.dt.float8e4, tag="junkd")
            junkh_sb = consts.tile([128, 64], BF16, tag="junkh")
            nc.gpsimd.memset(w_sb["wq"][:, 0, 0:64], 0.0)
            nc.gpsimd.memset(xhts[1][:, 0, 0:64], 0.0)
            nc.gpsimd.memset(dummy_sb[:], 0.0)
            nc.gpsimd.tensor_scalar_add(dummy_sb[:], dummy_sb[:], 1.0)
            nc.gpsimd.tensor_copy(junkh_sb[:], xhts[1][:, 0, 0:64])
            nc.gpsimd.tensor_copy(junk_sb[:], w_sb["wq"][:, 0, 0:64])
            for name in ("wq", "wk", "wv", "wp"):
                nc.gpsimd.dma_start(w_sb[name][:], w_ext[name][:])
            for hf in range(2):
                nc.gpsimd.dma_start(xhts[1][:, 2 * hf:2 * hf + 2, :],
                                    xh_ext[1, :, 2 * hf:2 * hf + 2, :])
            for b in range(BPC):
                for t in range(CT):
                    nc.gpsimd.dma_start(xts[b][t][:], x_ext[b, :, t, :])

            # PE warm-up: throwaway matmuls fill the initial DMA wait so the
            # HAM clock gate is already released (2.4 GHz) when the real
            # matmuls start (a >3.4us PE idle window re-throttles it).
            wu_sb = consts.tile([128, 512], BF16, tag="wu")
            nc.vector.memset(wu_sb[:], 0.0)
            ps_wu = psv.tile([128, 512], F32, tag="vmm", name="ps_warm")
            for i in range(26):
                nc.tensor.matmul(ps_wu[:], wu_sb[:, 0:128], wu_sb[:],
                                 start=(i == 0), stop=(i == 25))
            nc.vector.tensor_copy(wu_sb[:, 0:4], ps_wu[:, 0:4])

            def gn(b):
                # per-channel stats for all 4 channel tiles, then ONE
                # group-combine matmul and ONE 4-wide rsqrt chain
                mv = smallp.tile([128, CT, 2], F32, tag="mv", name=f"mv{b}")
                for t in range(CT):
                    stats = smallp.tile([128, 2, 6], F32, tag="stats",
                                        name=f"st{b}_{t}")
                    nc.vector.bn_stats(stats[:, 0, :], xhts[b][:, t, 0:512])
                    nc.vector.bn_stats(stats[:, 1, :], xhts[b][:, t, 512:1024])
                    nc.vector.bn_aggr(mv[:, t, :], stats[:])
                # s_all[:, 0, t]=mean_t, s_all[:, 1, t]=E[x^2]_t
                s_all = smallp.tile([128, 2, CT], F32, tag="s_all", name=f"s{b}")
                nc.vector.tensor_copy(s_all[:, 0, :], mv[:, :, 0])
                nc.vector.tensor_tensor(s_all[:, 1, :], mv[:, :, 0], mv[:, :, 0],
                                        Alu.mult)
                nc.vector.tensor_tensor(s_all[:, 1, :], s_all[:, 1, :],
                                        mv[:, :, 1], Alu.add)
                gs = psg.tile([128, 2, CT], F32, tag="gs", name=f"gs{b}")
                nc.tensor.matmul(gs[:], sel_sb[:], s_all[:], start=True, stop=True)
                gsb = smallp.tile([128, 2, CT], F32, tag="gsb", name=f"gb{b}")
                nc.vector.tensor_copy(gsb[:], gs[:])
                ab = smallp.tile([128, 4, CT], F32, tag="ab", name=f"ab{b}")
                va = ab[:, 0, :]         # var
                vp_ = ab[:, 1, :]        # var + eps
                y = ab[:, 2, :]
                tmp = ab[:, 3, :]
                nc.vector.tensor_tensor(va, gsb[:, 0, :], gsb[:, 0, :], Alu.mult)
                nc.vector.tensor_tensor(va, gsb[:, 1, :], va, Alu.subtract)
                # rstd = rsqrt(var+eps) entirely on DVE (fast-inverse-sqrt seed
                # + 2 Newton steps) so the scalar engine's activation tables
                # never leave the exp set (table reloads are 2.7us each).
                nc.vector.tensor_scalar_add(vp_, va, EPS)
                nc.vector.tensor_scalar(y.bitcast(I32), vp_.bitcast(I32), 1,
                                        None, Alu.arith_shift_right)
                nc.vector.tensor_tensor(y.bitcast(I32),
                                        magic_sb[:].to_broadcast([128, CT]),
                                        y.bitcast(I32), Alu.subtract)
                for _ in range(2):  # Newton: y *= 1.5 - 0.5*v*y^2
                    nc.vector.tensor_tensor(tmp, y, y, Alu.mult)
                    nc.vector.tensor_tensor(tmp, tmp, vp_, Alu.mult)
                    nc.vector.tensor_scalar(tmp, tmp, -0.5, 1.5, Alu.mult,
                                            Alu.add)
                    nc.vector.tensor_tensor(y, y, tmp, Alu.mult)
                a_all = ab[:, 0, :]      # reuse var slot: a = rstd*gamma
                bsh = ab[:, 3, :]
                nc.vector.tensor_tensor(a_all, y, vec_sb["gamma"][:], Alu.mult)
                nc.vector.tensor_tensor(bsh, gsb[:, 0, :], a_all, Alu.mult)
                nc.vector.tensor_tensor(bsh, vec_sb["beta"][:], bsh, Alu.subtract)
                # hn eviction split DVE/ACT to halve the serial latency on
                # the GN -> QKV critical path
                hn_sb = hnp.tile([128, CT, N], FP8, tag="hn", name=f"hn{b}")
                for t in range(CT):
                    nc.vector.tensor_scalar(hn_sb[:, t, :], xhts[b][:, t, :],
                                            ab[:, 0, t:t + 1],
                                            ab[:, 3, t:t + 1],
                                            Alu.mult, Alu.add)
                return hn_sb

            def make_xb(b):
                # xb = x + bias' on ACT; proj eviction is then psum*2^-11 + xb
                xb = xbp.tile([128, CT, N], F32, tag="xb", name=f"xb{b}")
                for t in range(CT):
                    nc.scalar.activation(xb[:, t, :], xts[b][t][:], Act.Identity,
                                         bias=vec_sb["bpp"][:, t:t + 1])
                return xb

            def qkv(b, hn_sb):
                q_sb = qkp.tile([128, CT, N], FP8, tag="q", name=f"q{b}")
                k_sb = qkp.tile([128, CT, N], FP8, tag="k", name=f"k{b}")
                for t in range(CT):
                    ts = slice(t * 128, (t + 1) * 128)
                    ps = psq.tile([128, N], F32, tag="mm", name=f"psq{b}_{t}")
                    for a in range(2):
                        for ch in range(NCH):
                            cs = slice(ch * 512, (ch + 1) * 512)
                            nc.tensor.matmul(
                                ps[:, cs], w_sb["wq"][:, 2 * a:2 * a + 2, ts],
                                hn_sb[:, 2 * a:2 * a + 2, cs],
                                start=(a == 0), stop=(a == 1), perf_mode=DR)
                    nc.scalar.activation(q_sb[:, t, :], ps[:], Act.Identity,
                                         bias=vec_sb["bq"][:, t:t + 1])
                    ps2 = psq.tile([128, N], F32, tag="mm", name=f"psk{b}_{t}")
                    for a in range(2):
                        for ch in range(NCH):
                            cs = slice(ch * 512, (ch + 1) * 512)
                            nc.tensor.matmul(
                                ps2[:, cs], w_sb["wk"][:, 2 * a:2 * a + 2, ts],
                                hn_sb[:, 2 * a:2 * a + 2, cs],
                                start=(a == 0), stop=(a == 1), perf_mode=DR)
                    nc.scalar.activation(k_sb[:, t, :], ps2[:], Act.Identity,
                                         bias=vec_sb["bk"][:, t:t + 1])
                # V, transposed: vT[n, c] = 16*v (no bias; folded into bpp).
                # Evictions alternate DVE/ACT to balance engine load.
                vT_sb = vp.tile([128, NT, 512], FP8, tag="vT", name=f"vT{b}")
                for jt in range(NT):
                    js = slice(jt * 128, (jt + 1) * 128)
                    ps = psv.tile([128, 512], F32, tag="vmm", name=f"psv{b}_{jt}")
                    for a in range(2):
                        nc.tensor.matmul(
                            ps[:], hn_sb[:, 2 * a:2 * a + 2, js],
                            w_sb["wv"][:, 2 * a:2 * a + 2, :],
                            start=(a == 0), stop=(a == 1), perf_mode=DR)
                    if jt % 2 == 0:
                        nc.vector.tensor_copy(vT_sb[:, jt, :], ps[:])
                    else:
                        nc.scalar.copy(vT_sb[:, jt, :], ps[:])
                return q_sb, k_sb, vT_sb

            def st_exp(b, q_sb, k_sb):
                e_sb = ep.tile([128, NT, N], FP8, tag="e", name=f"e{b}")
                for jt in range(NT):
                    js = slice(jt * 128, (jt + 1) * 128)
                    ps = psq.tile([128, N], F32, tag="mm", name=f"pss{b}_{jt}")
                    for a in range(2):
                        for ch in range(NCH):
                            cs = slice(ch * 512, (ch + 1) * 512)
                            nc.tensor.matmul(
                                ps[:, cs], k_sb[:, 2 * a:2 * a + 2, js],
                                q_sb[:, 2 * a:2 * a + 2, cs],
                                start=(a == 0), stop=(a == 1), perf_mode=DR)
                    # e = exp(q.k * scale - 1); the -1 keeps max e < 240 (fp8)
                    nc.scalar.activation(e_sb[:, jt, :], ps[:], Act.Exp,
                                         scale=SCALE / 256.0,
                                         bias=negone_sb[:])
                return e_sb

            def sums_recip(b, e_sb):
                # denominators: ones(=4) DR matmul straight over the e tiles
                sums_sb = rp.tile([1, N], F32, tag="sums", name=f"sm{b}")
                for ch in range(NCH):
                    cs = slice(ch * 512, (ch + 1) * 512)
                    ps1 = psg.tile([1, 512], F32, tag="onesum", name=f"os{b}_{ch}")
                    for a in range(4):
                        nc.tensor.matmul(ps1[:], ones_sb[:, :, 0:1],
                                         e_sb[:, 2 * a:2 * a + 2, cs],
                                         start=(a == 0), stop=(a == 3),
                                         perf_mode=DR)
                    nc.vector.tensor_copy(sums_sb[:, cs], ps1[:])
                recip_sb = rp.tile([1, N], F32, tag="recip", name=f"rc{b}")
                nc.vector.reciprocal_approx_fast(recip_sb[:], sums_sb[:])
                recipb_sb = rp.tile([128, N], F32, tag="recipb", name=f"rb{b}")
                nc.gpsimd.partition_broadcast(recipb_sb[:], recip_sb[:])
                return recipb_sb

            def pv(b, vT_sb, e_sb, ou_sb):
                # chunk-outer so each 512-chunk's ACT eviction overlaps the
                # next chunk's matmuls (normalization happens at proj evict,
                # keeping the recip chain off the PE critical path)
                for ct in range(CT):
                    ts = slice(ct * 128, (ct + 1) * 128)
                    ps = psq.tile([128, N], F32, tag="mm", name=f"pso{b}_{ct}")
                    for ch in range(NCH):
                        cs = slice(ch * 512, (ch + 1) * 512)
                        for a in range(4):
                            nc.tensor.matmul(
                                ps[:, cs], vT_sb[:, 2 * a:2 * a + 2, ts],
                                e_sb[:, 2 * a:2 * a + 2, cs],
                                start=(a == 0), stop=(a == 3), perf_mode=DR)
                        nc.scalar.activation(ou_sb[:, ct, cs], ps[:, cs],
                                             Act.Copy, scale=1.0 / 64.0)

            def proj(b, ou_sb, recipb_sb, xb):
                # [128,512] psums from the psv pool: no contention with the
                # next batch's qkv on psq. Eviction+store per 512-chunk keeps
                # the kernel tail after the last matmul to one chunk.
                for ot in range(CT):
                    ts = slice(ot * 128, (ot + 1) * 128)
                    o_sb = outp.tile([128, N], F32, tag="o", name=f"o{b}_{ot}")
                    for ch in range(NCH):
                        cs = slice(ch * 512, (ch + 1) * 512)
                        ps = psv.tile([128, 512], F32, tag="vmm",
                                      name=f"psp{b}_{ot}_{ch}")
                        for a in range(2):
                            nc.tensor.matmul(
                                ps[:], w_sb["wp"][:, 2 * a:2 * a + 2, ts],
                                ou_sb[:, 2 * a:2 * a + 2, cs],
                                start=(a == 0), stop=(a == 1), perf_mode=DR)
                        nc.vector.tensor_tensor(o_sb[:, cs], ps[:],
                                                recipb_sb[:, cs], Alu.mult)
                        nc.vector.tensor_tensor(o_sb[:, cs], o_sb[:, cs],
                                                xb[:, ot, cs], Alu.add)
                        nc.sync.dma_start(out_ext[b, :, ot, cs], o_sb[:, cs])

            # ---- software pipeline over the two batch items ----
            h0 = gn(0)
            q0, k0, v0 = qkv(0, h0)
            h1 = gn(1)
            e0 = st_exp(0, q0, k0)
            xb0 = make_xb(0)
            r0 = sums_recip(0, e0)
            o0 = oup.tile([128, CT, N], FP8, tag="ou", name="ou0")
            pv(0, v0, e0, o0)
            xb1 = make_xb(1)
            q1, k1, v1 = qkv(1, h1)
            proj(0, o0, r0, xb0)
            e1 = st_exp(1, q1, k1)
            r1 = sums_recip(1, e1)
            o1 = oup.tile([128, CT, N], FP8, tag="ou", name="ou1")
            pv(1, v1, e1, o1)
            proj(1, o1, r1, xb1)

    nc.compile()
    return nc


def _prep_vec(v):
    # [C] f32 -> [128, CT] with v_sb[p, t] = v[t*128 + p]
    return np.ascontiguousarray(
        np.asarray(v, dtype=np.float32).reshape(CT, 128).T)


def _prep_w8(w):
    # [C, C] (out, in) -> lhsT layout [128, CT, 512] fp8, scaled by 16:
    # w_sb[p, it, o] = 16 * w[o, it*128 + p]
    wT = np.asarray(w, dtype=np.float32).T * 16.0
    return np.ascontiguousarray(
        wT.reshape(CT, 128, C).transpose(1, 0, 2).astype(ml_dtypes.float8_e4m3))


def kernel(x, gamma, beta, wq, bq, wk, bk, wv, bv, wp, bp):
    from concourse.bass_utils import run_bass_kernel_spmd

    nc = _CACHE.get("nc")
    if nc is None:
        nc = _CACHE["nc"] = _build_bass()

    x = np.asarray(x, dtype=np.float32)
    # [16, C, H, W] -> [16, 128, CT, N]
    xr = np.ascontiguousarray(
        x.reshape(B_TOT, CT, 128, N).transpose(0, 2, 1, 3))

    bpp = np.asarray(wp, np.float32) @ np.asarray(bv, np.float32) \
        + np.asarray(bp, np.float32)
    sel = np.kron(np.eye(128 // GS, dtype=np.float32),
                  np.full((GS, GS), 1.0 / GS, dtype=np.float32))
    xh = xr.astype(ml_dtypes.bfloat16)
    common = {
        "wq": _prep_w8(wq), "wk": _prep_w8(wk), "wv": _prep_w8(wv),
        "wp": _prep_w8(wp),
        "gamma": _prep_vec(gamma), "beta": _prep_vec(beta),
        "bq": _prep_vec(16.0 * np.asarray(bq, np.float32)),
        "bk": _prep_vec(16.0 * np.asarray(bk, np.float32)),
        "bpp": _prep_vec(bpp),
        "sel": sel,
        "ones": np.full((128, 2, 16), 4.0, dtype=ml_dtypes.float8_e4m3),
    }
    in_maps = [
        {"x": np.ascontiguousarray(xr[c * BPC:(c + 1) * BPC]),
         "xh": np.ascontiguousarray(xh[c * BPC:(c + 1) * BPC]), **common}
        for c in range(NCORES)
    ]
    res = run_bass_kernel_spmd(nc, in_maps, core_ids=list(range(NCORES)))
    # [BPC, 128, CT, N] per core -> [16, C, H, W]
    out = np.concatenate([r["out"] for r in res.results], axis=0)
    return np.ascontiguousarray(
        out.transpose(0, 2, 1, 3)).reshape(B_TOT, C, H, W)
